# revision 6
# baseline (speedup 1.0000x reference)
"""Trainium2 Bass kernel for a single-step attention decoder RNN (GRU).

Model (batch=1): Bahdanau additive attention over S=512 encoder states,
input projection, one GRU step (H=2048), and a log-softmax output
projection over V=50257 vocab.

Sharding across 8 NeuronCores (tensor-parallel):
  - attention / fc / GRU sharded over the hidden dim (256 rows per core),
  - out projection + embedding table sharded over vocab / columns,
  - four tiny AllGathers stitch the pieces (scores+embedding, x, h_new,
    log-softmax normalizer stats).

All weight matrices are passed to the device pre-transposed (host-side
layout change only) so that every matvec runs on the TensorEngine with
the vector as the 128x1 stationary operand and the weight tile as the
128x512 moving operand -- no on-device transposes of large tensors.
"""

import functools
import os
import sys

for _p in ("/opt/trn_rl_repo", "/root/.axon_site/_ro/trn_rl_repo"):
    if os.path.isdir(_p):
        if _p not in sys.path:
            sys.path.insert(0, _p)
        break

import numpy as np

import concourse.bass as bass
import concourse.mybir as mybir
import concourse.tile as tile
from concourse import bacc
from concourse.masks import make_identity

NC = 8
V, H, S = 50257, 2048, 512
HC = H // NC              # 256: per-core hidden slice
GC = 3 * HC               # 768: per-core GRU gate rows
VS = (V + NC - 1) // NC   # 6283: per-core vocab shard (last core padded)
VBLK = [512] * 12 + [VS - 12 * 512]   # v-blocks within a core's shard
NEG_BIG = -1.0e30

FP = mybir.dt.float32
AX = mybir.AxisListType
ALU = mybir.AluOpType
ACTF = mybir.ActivationFunctionType


def _build_nc():
    nc = bacc.Bacc("TRN2", target_bir_lowering=False, debug=False, num_devices=NC)

    # ---- per-core DRAM inputs ----
    tok = nc.dram_tensor("tok", [1], mybir.dt.int32, kind="ExternalInput")
    h_full = nc.dram_tensor("h_full", [H], FP, kind="ExternalInput")
    h_sl = nc.dram_tensor("h_sl", [HC], FP, kind="ExternalInput")
    encT = nc.dram_tensor("encT", [H, S], FP, kind="ExternalInput")
    enc_nat = nc.dram_tensor("enc_nat", [S, H], FP, kind="ExternalInput")
    attn_wt_sl = nc.dram_tensor("attn_wt_sl", [2 * H, HC], FP, kind="ExternalInput")
    attn_b_sl = nc.dram_tensor("attn_b_sl", [HC], FP, kind="ExternalInput")
    v_sl = nc.dram_tensor("v_sl", [HC], FP, kind="ExternalInput")
    fc_wt_sl = nc.dram_tensor("fc_wt_sl", [2 * H, HC], FP, kind="ExternalInput")
    fc_b_sl = nc.dram_tensor("fc_b_sl", [HC], FP, kind="ExternalInput")
    w_ih_t_sl = nc.dram_tensor("w_ih_t_sl", [H, GC], FP, kind="ExternalInput")
    w_hh_t_sl = nc.dram_tensor("w_hh_t_sl", [H, GC], FP, kind="ExternalInput")
    b_ih_sl = nc.dram_tensor("b_ih_sl", [GC], FP, kind="ExternalInput")
    b_hh_sl = nc.dram_tensor("b_hh_sl", [GC], FP, kind="ExternalInput")
    out_wt_sl = nc.dram_tensor("out_wt_sl", [H, VS], FP, kind="ExternalInput")
    out_b_sl = nc.dram_tensor("out_b_sl", [VS], FP, kind="ExternalInput")
    e_sl = nc.dram_tensor("e_sl", [V, HC], FP, kind="ExternalInput")

    # ---- per-core DRAM outputs ----
    logp_sl = nc.dram_tensor("logp_sl", [VS], FP, kind="ExternalOutput")
    h_new_out = nc.dram_tensor("h_new_out", [H], FP, kind="ExternalOutput")
    attn_w_out = nc.dram_tensor("attn_w_out", [S], FP, kind="ExternalOutput")

    RG = [list(range(NC))]

    with tile.TileContext(nc) as tc:
        with (
            tc.tile_pool(name="persist", bufs=1) as pp,
            tc.tile_pool(name="dram", bufs=1, space="DRAM") as dp,
        ):
            # ---------- small loads ----------
            idx_sb = pp.tile([2, 1], mybir.dt.int32)
            nc.sync.dma_start(idx_sb[0:1, 0:1], tok.ap().rearrange("(a x) -> a x", a=1))
            nc.sync.dma_start(idx_sb[1:2, 0:1], tok.ap().rearrange("(a x) -> a x", a=1))

            emb2_sb = pp.tile([2, HC], FP)
            nc.gpsimd.indirect_dma_start(
                out=emb2_sb[:],
                out_offset=None,
                in_=e_sl.ap(),
                in_offset=bass.IndirectOffsetOnAxis(ap=idx_sb[:, 0:1], axis=0),
            )

            # h in two layouts: (f p)->p f gives chunk k = column k
            h_pf = pp.tile([128, H // 128], FP)
            nc.sync.dma_start(h_pf[:], h_full.ap().rearrange("(f p) -> p f", p=128))
            h_sl_sb = pp.tile([1, HC], FP)
            nc.sync.dma_start(h_sl_sb[:], h_sl.ap().rearrange("(a x) -> a x", a=1))

            attn_b_pf = pp.tile([128, HC // 128], FP)
            nc.sync.dma_start(
                attn_b_pf[:], attn_b_sl.ap().rearrange("(f p) -> p f", p=128)
            )
            v_pf = pp.tile([128, HC // 128], FP)
            nc.sync.dma_start(v_pf[:], v_sl.ap().rearrange("(f p) -> p f", p=128))
            fc_b_sb = pp.tile([1, HC], FP)
            nc.sync.dma_start(fc_b_sb[:], fc_b_sl.ap().rearrange("(a x) -> a x", a=1))
            b_ih_sb = pp.tile([1, GC], FP)
            nc.sync.dma_start(b_ih_sb[:], b_ih_sl.ap().rearrange("(a x) -> a x", a=1))
            b_hh_sb = pp.tile([1, GC], FP)
            nc.sync.dma_start(b_hh_sb[:], b_hh_sl.ap().rearrange("(a x) -> a x", a=1))

            ident = pp.tile([128, 128], FP)
            make_identity(nc, ident[:])
            ones8 = pp.tile([8, 1], FP)
            nc.vector.memset(ones8[:], 1.0)

            # ---------- attention ----------
            with (
                tc.tile_pool(name="attn_w_pool", bufs=1) as awp,
                tc.tile_pool(name="encT_pool", bufs=1) as etp,
                tc.tile_pool(name="enc_nat_pool", bufs=1) as enp,
                tc.tile_pool(name="psA", bufs=1, space="PSUM") as psA,
            ):
                encT_sb = []
                for hcn in range(16):
                    t = etp.tile([128, S], FP, name=f"encT_{hcn}", tag=f"encT_{hcn}")
                    nc.sync.dma_start(t[:], encT.ap()[hcn * 128:(hcn + 1) * 128, :])
                    encT_sb.append(t)
                awt_sb = []
                for rc in range(32):
                    t = awp.tile([128, HC], FP, name=f"awt_{rc}", tag=f"awt_{rc}")
                    nc.sync.dma_start(t[:], attn_wt_sl.ap()[rc * 128:(rc + 1) * 128, :])
                    awt_sb.append(t)
                enc_sb = []
                for st in range(4):
                    t = enp.tile([128, H], FP, name=f"enc_{st}", tag=f"enc_{st}")
                    nc.sync.dma_start(t[:], enc_nat.ap()[st * 128:(st + 1) * 128, :])
                    enc_sb.append(t)

                # energies^T[j, s] (j = this core's 256-slice of H)
                energ_sb = pp.tile([128, 2, S], FP)
                ps_sc = psA.tile([1, S], FP, tag="ps_sc")
                for jt in range(2):
                    ps_e = psA.tile([128, S], FP, tag="ps_e")
                    for hcn in range(16):
                        nc.tensor.matmul(
                            ps_e[:],
                            lhsT=awt_sb[16 + hcn][:, jt * 128:(jt + 1) * 128],
                            rhs=encT_sb[hcn][:],
                            start=(hcn == 0),
                            stop=(hcn == 15),
                        )
                    ps_h = psA.tile([128, 1], FP, tag="ps_h")
                    for hcn in range(16):
                        nc.tensor.matmul(
                            ps_h[:],
                            lhsT=awt_sb[hcn][:, jt * 128:(jt + 1) * 128],
                            rhs=h_pf[:, hcn:hcn + 1],
                            start=(hcn == 0),
                            stop=(hcn == 15),
                        )
                    bias_jt = pp.tile([128, 1], FP, name=f"bias_{jt}", tag=f"bias_{jt}")
                    nc.vector.tensor_add(bias_jt[:], ps_h[:], attn_b_pf[:, jt:jt + 1])
                    nc.scalar.activation(
                        energ_sb[:, jt, :], ps_e[:], ACTF.Tanh, bias=bias_jt[:]
                    )
                    # partial scores over this j-tile
                    nc.tensor.matmul(
                        ps_sc[:],
                        lhsT=v_pf[:, jt:jt + 1],
                        rhs=energ_sb[:, jt, :],
                        start=(jt == 0),
                        stop=(jt == 1),
                    )

                # ---------- AllGather #1: [partial scores (512) | emb slice (256)] ----------
                sc_part = pp.tile([1, S], FP)
                nc.vector.tensor_copy(sc_part[:], ps_sc[:])
                cc1_in = dp.tile([S + HC], FP)
                cc1_out = dp.tile([NC, S + HC], FP, addr_space="Shared")
                nc.sync.dma_start(cc1_in[0:S].rearrange("(a x) -> a x", a=1), sc_part[:])
                nc.sync.dma_start(
                    cc1_in[S:S + HC].rearrange("(a x) -> a x", a=1), emb2_sb[0:1, :]
                )
                nc.gpsimd.collective_compute(
                    "AllGather", ALU.bypass, replica_groups=RG,
                    ins=[cc1_in.opt()], outs=[cc1_out.opt()],
                )
                ag1_sb = pp.tile([NC, S + HC], FP)
                nc.sync.dma_start(ag1_sb[:], cc1_out[:])

                # scores = sum over cores of partials
                ps_scf = psA.tile([1, S], FP, tag="ps_scf")
                nc.tensor.matmul(
                    ps_scf[:], lhsT=ones8[:], rhs=ag1_sb[:, 0:S],
                    start=True, stop=True,
                )
                # embedding -> column-chunk layout (chunk k of 128 = col k//2 of A/B)
                emb_cols = []
                for half in range(2):
                    ps_t = psA.tile([128, NC], FP, tag="ps_embT")
                    nc.tensor.transpose(
                        ps_t[:],
                        ag1_sb[:, S + half * 128: S + (half + 1) * 128],
                        ident[:NC, :NC],
                    )
                    cols = pp.tile([128, NC], FP, name=f"embc_{half}", tag=f"embc_{half}")
                    nc.vector.tensor_copy(cols[:], ps_t[:])
                    emb_cols.append(cols)

                # ---------- softmax ----------
                smax = pp.tile([1, 1], FP)
                nc.vector.reduce_max(smax[:], ps_scf[:], axis=AX.X)
                neg_smax = pp.tile([1, 1], FP)
                nc.vector.tensor_scalar_mul(neg_smax[:], smax[:], -1.0)
                attnw_sb = pp.tile([1, S], FP)
                ssum = pp.tile([1, 1], FP)
                nc.scalar.activation(
                    attnw_sb[:], ps_scf[:], ACTF.Exp,
                    bias=neg_smax[:], accum_out=ssum[:],
                )
                rsum = pp.tile([1, 1], FP)
                nc.vector.reciprocal(rsum[:], ssum[:])
                nc.vector.tensor_scalar_mul(attnw_sb[:], attnw_sb[:], rsum[:])
                nc.sync.dma_start(attn_w_out.ap().rearrange("(a x) -> a x", a=1), attnw_sb[:])

                # attn_w to per-partition chunks: (1,512) -> (128,4)
                attnw_pf = pp.tile([128, 4], FP)
                for st in range(4):
                    ps_t2 = psA.tile([128, 1], FP, tag="ps_awT")
                    nc.tensor.transpose(
                        ps_t2[:], attnw_sb[:, st * 128:(st + 1) * 128], ident[:1, :1]
                    )
                    nc.vector.tensor_copy(attnw_pf[:, st:st + 1], ps_t2[:])

                # ---------- context (full H, replicated) ----------
                ps_ctx = psA.tile([128, 16], FP, tag="ps_ctx")
                for jt in range(16):
                    for st in range(4):
                        nc.tensor.matmul(
                            ps_ctx[:, jt:jt + 1],
                            lhsT=enc_sb[st][:, jt * 128:(jt + 1) * 128],
                            rhs=attnw_pf[:, st:st + 1],
                            start=(st == 0),
                            stop=(st == 3),
                        )
                ctx_pf = pp.tile([128, 16], FP)
                nc.vector.tensor_copy(ctx_pf[:], ps_ctx[:])

            # ---------- fc: x = [emb | ctx] @ fc_W.T + fc_b (this core's 256 outs) ----------
            with (
                tc.tile_pool(name="fc_pool", bufs=1) as fcp,
                tc.tile_pool(name="psB", bufs=1, space="PSUM") as psB,
            ):
                fcw_sb = []
                for k in range(32):
                    t = fcp.tile([128, HC], FP, name=f"fcw_{k}", tag=f"fcw_{k}")
                    nc.sync.dma_start(t[:], fc_wt_sl.ap()[k * 128:(k + 1) * 128, :])
                    fcw_sb.append(t)
                ps_x = psB.tile([1, HC], FP, tag="ps_x")
                for k in range(32):
                    if k < 16:
                        chunk = emb_cols[k % 2][:, k // 2: k // 2 + 1]
                    else:
                        chunk = ctx_pf[:, k - 16: k - 15]
                    nc.tensor.matmul(
                        ps_x[:], lhsT=chunk, rhs=fcw_sb[k][:],
                        start=(k == 0), stop=(k == 31),
                    )
                x_sb = pp.tile([1, HC], FP)
                nc.vector.tensor_add(x_sb[:], ps_x[:], fc_b_sb[:])

            # ---------- AllGather #2: x ----------
            psC_ctx = tc.tile_pool(name="psC", bufs=1, space="PSUM")
            psC = psC_ctx.__enter__()
            cc2_in = dp.tile([HC], FP)
            cc2_out = dp.tile([NC, HC], FP, addr_space="Shared")
            nc.sync.dma_start(cc2_in[:].rearrange("(a x) -> a x", a=1), x_sb[:])
            nc.gpsimd.collective_compute(
                "AllGather", ALU.bypass, replica_groups=RG,
                ins=[cc2_in.opt()], outs=[cc2_out.opt()],
            )
            ag2_sb = pp.tile([NC, HC], FP)
            nc.sync.dma_start(ag2_sb[:], cc2_out[:])
            x_cols = []
            for half in range(2):
                ps_t3 = psC.tile([128, NC], FP, tag="ps_xT")
                nc.tensor.transpose(
                    ps_t3[:], ag2_sb[:, half * 128:(half + 1) * 128], ident[:NC, :NC]
                )
                cols = pp.tile([128, NC], FP, name=f"xc_{half}", tag=f"xc_{half}")
                nc.vector.tensor_copy(cols[:], ps_t3[:])
                x_cols.append(cols)

            # ---------- GRU gates (this core's 256-slice of each gate) ----------
            with (
                tc.tile_pool(name="whh_pool", bufs=4) as whhp,
                tc.tile_pool(name="wih_pool", bufs=4) as wihp,
            ):
                ps_gh_a = psC.tile([1, 512], FP, tag="ps_gh_a")
                ps_gh_b = psC.tile([1, GC - 512], FP, tag="ps_gh_b")
                for hcn in range(16):
                    t = whhp.tile([128, GC], FP, tag="whh")
                    nc.sync.dma_start(t[:], w_hh_t_sl.ap()[hcn * 128:(hcn + 1) * 128, :])
                    nc.tensor.matmul(
                        ps_gh_a[:], lhsT=h_pf[:, hcn:hcn + 1], rhs=t[:, 0:512],
                        start=(hcn == 0), stop=(hcn == 15),
                    )
                    nc.tensor.matmul(
                        ps_gh_b[:], lhsT=h_pf[:, hcn:hcn + 1], rhs=t[:, 512:GC],
                        start=(hcn == 0), stop=(hcn == 15),
                    )
                ps_gi_a = psC.tile([1, 512], FP, tag="ps_gi_a")
                ps_gi_b = psC.tile([1, GC - 512], FP, tag="ps_gi_b")
                for hcn in range(16):
                    t = wihp.tile([128, GC], FP, tag="wih")
                    nc.sync.dma_start(t[:], w_ih_t_sl.ap()[hcn * 128:(hcn + 1) * 128, :])
                    chunk = x_cols[hcn % 2][:, hcn // 2: hcn // 2 + 1]
                    nc.tensor.matmul(
                        ps_gi_a[:], lhsT=chunk, rhs=t[:, 0:512],
                        start=(hcn == 0), stop=(hcn == 15),
                    )
                    nc.tensor.matmul(
                        ps_gi_b[:], lhsT=chunk, rhs=t[:, 512:GC],
                        start=(hcn == 0), stop=(hcn == 15),
                    )

                gi_sb = pp.tile([1, GC], FP)
                nc.vector.tensor_add(gi_sb[:, 0:512], ps_gi_a[:], b_ih_sb[:, 0:512])
                nc.vector.tensor_add(gi_sb[:, 512:GC], ps_gi_b[:], b_ih_sb[:, 512:GC])
                gh_sb = pp.tile([1, GC], FP)
                nc.vector.tensor_add(gh_sb[:, 0:512], ps_gh_a[:], b_hh_sb[:, 0:512])
                nc.vector.tensor_add(gh_sb[:, 512:GC], ps_gh_b[:], b_hh_sb[:, 512:GC])

            psC_ctx.__exit__(None, None, None)

            rz_in = pp.tile([1, 512], FP)
            nc.vector.tensor_add(rz_in[:], gi_sb[:, 0:512], gh_sb[:, 0:512])
            rz_sb = pp.tile([1, 512], FP)
            nc.scalar.activation(rz_sb[:], rz_in[:], ACTF.Sigmoid)
            n_in = pp.tile([1, HC], FP)
            nc.vector.tensor_mul(n_in[:], rz_sb[:, 0:HC], gh_sb[:, 512:GC])
            nc.vector.tensor_add(n_in[:], n_in[:], gi_sb[:, 512:GC])
            n_sb = pp.tile([1, HC], FP)
            nc.scalar.activation(n_sb[:], n_in[:], ACTF.Tanh)
            # h_new = n + z*(h - n)
            hmn = pp.tile([1, HC], FP)
            nc.vector.tensor_sub(hmn[:], h_sl_sb[:], n_sb[:])
            nc.vector.tensor_mul(hmn[:], hmn[:], rz_sb[:, HC:512])
            hn_sl = pp.tile([1, HC], FP)
            nc.vector.tensor_add(hn_sl[:], n_sb[:], hmn[:])

            # ---------- AllGather #3: h_new ----------
            cc3_in = dp.tile([HC], FP)
            cc3_out = dp.tile([NC, HC], FP, addr_space="Shared")
            nc.sync.dma_start(cc3_in[:].rearrange("(a x) -> a x", a=1), hn_sl[:])
            nc.gpsimd.collective_compute(
                "AllGather", ALU.bypass, replica_groups=RG,
                ins=[cc3_in.opt()], outs=[cc3_out.opt()],
            )
            psD_ctx = tc.tile_pool(name="psD", bufs=1, space="PSUM")
            psD = psD_ctx.__enter__()
            ag3_sb = pp.tile([NC, HC], FP)
            nc.sync.dma_start(ag3_sb[:], cc3_out[:])
            nc.sync.dma_start(h_new_out.ap().rearrange("(p f) -> p f", p=NC), ag3_sb[:])
            hn_cols = []
            for half in range(2):
                ps_t4 = psD.tile([128, NC], FP, tag="ps_hnT")
                nc.tensor.transpose(
                    ps_t4[:], ag3_sb[:, half * 128:(half + 1) * 128], ident[:NC, :NC]
                )
                cols = pp.tile([128, NC], FP, name=f"hnc_{half}", tag=f"hnc_{half}")
                nc.vector.tensor_copy(cols[:], ps_t4[:])
                hn_cols.append(cols)

            # ---------- out projection: logits = out_W @ h_new + out_b ----------
            logits_sb = pp.tile([1, VS], FP)
            with tc.tile_pool(name="outw_pool", bufs=24) as owp:
                off = 0
                for vb, w in enumerate(VBLK):
                    ps_o = psD.tile([1, w], FP, tag="ps_o", bufs=4)
                    for hcn in range(16):
                        t = owp.tile([128, w], FP, tag="outw")
                        nc.sync.dma_start(
                            t[:],
                            out_wt_sl.ap()[hcn * 128:(hcn + 1) * 128, off:off + w],
                        )
                        chunk = hn_cols[hcn % 2][:, hcn // 2: hcn // 2 + 1]
                        nc.tensor.matmul(
                            ps_o[:], lhsT=chunk, rhs=t[:],
                            start=(hcn == 0), stop=(hcn == 15),
                        )
                    outb_blk = pp.tile([1, w], FP, tag="outb_blk", bufs=2)
                    nc.sync.dma_start(
                        outb_blk[:],
                        out_b_sl.ap()[off:off + w].rearrange("(a x) -> a x", a=1),
                    )
                    nc.vector.tensor_add(
                        logits_sb[:, off:off + w], ps_o[:], outb_blk[:]
                    )
                    off += w

            # ---------- log-softmax normalizer ----------
            m_c = pp.tile([1, 1], FP)
            nc.vector.reduce_max(m_c[:], logits_sb[:], axis=AX.X)
            neg_m = pp.tile([1, 1], FP)
            nc.vector.tensor_scalar_mul(neg_m[:], m_c[:], -1.0)
            s_part = pp.tile([1, len(VBLK)], FP)
            exp_scr = pp.tile([1, 512], FP)
            off2 = 0
            for vb, w in enumerate(VBLK):
                nc.scalar.activation(
                    exp_scr[:, 0:w], logits_sb[:, off2:off2 + w], ACTF.Exp,
                    bias=neg_m[:], accum_out=s_part[:, vb:vb + 1],
                )
                off2 += w
            s_c = pp.tile([1, 1], FP)
            nc.vector.reduce_sum(s_c[:], s_part[:], axis=AX.X)
            stats = pp.tile([1, 8], FP)
            nc.vector.memset(stats[:], 0.0)
            nc.vector.tensor_copy(stats[:, 0:1], m_c[:])
            nc.vector.tensor_copy(stats[:, 1:2], s_c[:])
            cc4_in = dp.tile([8], FP)
            cc4_out = dp.tile([NC, 8], FP, addr_space="Shared")
            nc.sync.dma_start(cc4_in[:].rearrange("(a x) -> a x", a=1), stats[:])
            nc.gpsimd.collective_compute(
                "AllGather", ALU.bypass, replica_groups=RG,
                ins=[cc4_in.opt()], outs=[cc4_out.opt()],
            )
            ag4_sb = pp.tile([NC, 8], FP)
            nc.sync.dma_start(ag4_sb[:], cc4_out[:])
            ps_m = psD.tile([1, NC], FP, tag="ps_m")
            nc.tensor.matmul(ps_m[:], lhsT=ag4_sb[:, 0:1], rhs=ident[:NC, :NC],
                             start=True, stop=True)
            ps_s = psD.tile([1, NC], FP, tag="ps_s")
            nc.tensor.matmul(ps_s[:], lhsT=ag4_sb[:, 1:2], rhs=ident[:NC, :NC],
                             start=True, stop=True)

            m_g = pp.tile([1, 1], FP)
            nc.vector.reduce_max(m_g[:], ps_m[:], axis=AX.X)
            neg_mg = pp.tile([1, 1], FP)
            nc.vector.tensor_scalar_mul(neg_mg[:], m_g[:], -1.0)
            expm = pp.tile([1, NC], FP)
            nc.scalar.activation(expm[:], ps_m[:], ACTF.Exp, bias=neg_mg[:])
            junk = pp.tile([1, NC], FP)
            z_g = pp.tile([1, 1], FP)
            nc.vector.tensor_mul(junk[:], expm[:], ps_s[:])
            nc.vector.reduce_sum(z_g[:], junk[:], axis=AX.X)
            logz = pp.tile([1, 1], FP)
            nc.scalar.activation(logz[:], z_g[:], ACTF.Ln)
            norm = pp.tile([1, 1], FP)
            nc.vector.tensor_add(norm[:], m_g[:], logz[:])
            neg_norm = pp.tile([1, 1], FP)
            nc.vector.tensor_scalar_mul(neg_norm[:], norm[:], -1.0)

            nc.vector.tensor_scalar_add(logits_sb[:], logits_sb[:], neg_norm[:])
            nc.sync.dma_start(logp_sl.ap().rearrange("(a x) -> a x", a=1), logits_sb[:])
            psD_ctx.__exit__(None, None, None)

    nc.compile()
    return nc


_NC_CACHE = None


def _get_nc():
    global _NC_CACHE
    if _NC_CACHE is None:
        _NC_CACHE = _build_nc()
    return _NC_CACHE


def _shard_inputs(
    input_tok, hidden, enc_outputs, E, attn_W, attn_b, v, fc_W, fc_b,
    W_ih, W_hh, b_ih, b_hh, out_W, out_b,
):
    f32 = lambda a: np.ascontiguousarray(a, dtype=np.float32)
    h = np.asarray(hidden, dtype=np.float32).reshape(H)
    tok = np.asarray(input_tok).reshape(1).astype(np.int32)
    enc = f32(np.asarray(enc_outputs))
    encT = f32(np.asarray(enc_outputs).T)
    attn_W = np.asarray(attn_W)
    fc_W = np.asarray(fc_W)
    W_ih, W_hh = np.asarray(W_ih), np.asarray(W_hh)
    b_ih, b_hh = np.asarray(b_ih), np.asarray(b_hh)
    out_W, out_b = np.asarray(out_W), np.asarray(out_b)
    E = np.asarray(E)

    in_maps = []
    for c in range(NC):
        j0, j1 = c * HC, (c + 1) * HC
        gsl = np.r_[0 * H + j0:0 * H + j1, 1 * H + j0:1 * H + j1, 2 * H + j0:2 * H + j1]
        r0 = c * VS
        r1 = min(V, r0 + VS)
        owt = np.zeros((H, VS), dtype=np.float32)
        owt[:, : r1 - r0] = out_W[r0:r1, :].T
        ob = np.full((VS,), NEG_BIG, dtype=np.float32)
        ob[: r1 - r0] = out_b[r0:r1]
        in_maps.append({
            "tok": tok,
            "h_full": h,
            "h_sl": f32(h[j0:j1]),
            "encT": encT,
            "enc_nat": enc,
            "attn_wt_sl": f32(attn_W[j0:j1, :].T),
            "attn_b_sl": f32(attn_b[j0:j1]),
            "v_sl": f32(v[j0:j1]),
            "fc_wt_sl": f32(fc_W[j0:j1, :].T),
            "fc_b_sl": f32(fc_b[j0:j1]),
            "w_ih_t_sl": f32(W_ih[gsl, :].T),
            "w_hh_t_sl": f32(W_hh[gsl, :].T),
            "b_ih_sl": f32(b_ih[gsl]),
            "b_hh_sl": f32(b_hh[gsl]),
            "out_wt_sl": owt,
            "out_b_sl": ob,
            "e_sl": f32(E[:, j0:j1]),
        })
    return in_maps


def kernel(**inputs):
    from concourse import bass_utils

    nc = _get_nc()
    in_maps = _shard_inputs(**inputs)
    res = bass_utils.run_bass_kernel_spmd(
        nc, in_maps, core_ids=list(range(NC))
    )
    r = res.results
    log_probs = np.concatenate([r[c]["logp_sl"] for c in range(NC)])[:V]
    h_new = r[0]["h_new_out"]
    attn_w = r[0]["attn_w_out"]
    return (
        log_probs[None, :].astype(np.float32),
        h_new[None, None, :].astype(np.float32),
        attn_w[None, None, :].astype(np.float32),
    )
